# revision 25
# baseline (speedup 1.0000x reference)
"""MoE FFN with hierarchical KV router — Trainium2 Bass kernel (8 NeuronCores).

Strategy (expert-parallel, per the sharding hint):
  * Host computes the router (l2-norm scores -> softmax over EPB=4 -> top-2 ->
    combine weights) and dispatches tokens by global expert id.
  * Expert e lives on core e//2. Each core runs three segments in the order
    [expert0, shared-slice, expert1] (the last segment has the shortest
    input->output chain, minimizing the kernel tail):
      expert segs: that expert's FFN over <=CAP_E tokens in fp8-e4m3
               (DoubleRow matmuls, 2x rate). Expert-path quantization error
               is damped by gate = sigmoid(-2) ~ 0.119 in the final combine.
      shared seg: H-split slice of the shared dense FFN in bf16: core pair
               p = c//2 owns tokens [512p, 512p+512); core 2p computes the
               H-half 0 partial, core 2p+1 the H-half 1 partial; the host
               adds the two partials. Halves the shared-weight replication
               (the chip-level HBM stream is the binding resource).
    out_seg = relu(x @ W1 + b1) @ W2 + b2 per token slot; host combines:
        y[tok] = gate*(w0*row_e0 + w1*row_e1) + (partial0[tok]+partial1[tok])
  * fp8 scaling: x*16, W1*256 -> psum = 4096*(x@W1); ReLU applies scale 1/256
    and emits h1_fp8 = 16*relu(x@W1); W2*256 -> psum2 = 4096*(h1@W2); the out
    op applies 16/4096 and emits fp8*16.

Fast path (all-zero biases, which setup_inputs always produces): see
_build_program_fast's docstring — tensor phases run [E0, E1-mm1, S, E1-mm2]
against an input stream [e0, e1 xT+W1, shared, e1 W2] so HBM jitter can't
starve the PE mid-kernel; all 8 PSUM banks are planned so each phase starts
in banks the previous phase freed earliest; drains are paired 2-bank
activations split scalar/vector; no bias DMA (zero tile); 8 full-duty
512-col dummy matmuls warm the HAM clock gate during the first input's
flight; no completion wait on the output DMAs (block-end drains + the
runtime teardown exceed their in-flight time).
A biased fallback (the previous program) is kept for nonzero-bias inputs.
"""
import sys

if "/opt/trn_rl_repo" not in sys.path:
    sys.path.insert(0, "/opt/trn_rl_repo")

import numpy as np
import ml_dtypes

N_BUCKET, EPB, TOPK, TAU = 4, 4, 2, 1.0
C, H = 512, 1024
E = N_BUCKET * EPB
KC, KH = C // 128, H // 128  # contraction blocks: 4, 8
N_CORES = 8
SX, S1, SHS, S2, SO = 16.0, 256.0, 16.0, 256.0, 16.0  # fp8 scales
F8 = ml_dtypes.float8_e4m3
BF16 = ml_dtypes.bfloat16
CAP_S = 512          # shared tokens per core (pair-owned, H-split)
KHS = KH // 2        # shared h1 blocks per core (H-half)
N_DUMMY = 8          # HAM warm-up: 512-col DR dummies, full PE duty (~3.4us)

_BUILD_CACHE = {}


def _elayout(CAP_E):
    EXT = 0
    EW1 = EXT + KC * CAP_E       # W1 col m*512 + k*128 + q (m-major)
    EW2 = EW1 + KC * H           # W2 col j*1024 + m2*256 + i*128 + c (DR pairs)
    ECOLS = EW2 + KH * C
    return EXT, EW1, EW2, ECOLS


# shared blob (H-split): xT | W1-half (m-major) | W2-half (k2-major)
SXT = 0
SW1 = KC * CAP_S                 # 2048
SW2 = SW1 + KC * (H // 2)        # +2048
SCOLS = SW2 + KHS * C            # +2048


def _build_program_fast(CAP_E):
    """Zero-bias program, v5: phase order [E0, E1-mm1, S, E1-mm2].

    The input stream is [e0, e1 xT+W1, shared, e1 W2], so every phase has
    >=1us of delivery margin when the tensor reaches it - HBM arbitration
    jitter between the paired cores can no longer starve the PE mid-kernel
    (starvation idles crossed the HAM window and re-gated the PE clock to
    1.2 GHz for whole segments).

    Banks: expert h1 block m -> bank m; shared h1 block m -> bank 4+m;
    E0/S out block m2 -> bank m2; E1 out block m2 -> bank 4+m2. Every
    phase starts in banks the previous phase freed earliest, and each
    mm2's banks are freed by drains that complete before its mm1 ends.
    h1 drains are engine-paired 2-bank activations (strided AP); out
    drains are one pair per engine, shipped as one DMA per engine half
    (E1's scalar half issues from the scalar HWDGE queue so the two tail
    DMAs overlap). HAM warm-up: 8 x 512-col DR dummies, full PE duty.
    No completion wait on the output DMAs: the block-end drains plus the
    runtime teardown (~8us of semaphore clears) far exceed the last DMA's
    in-flight time.
    """
    from contextlib import ExitStack

    import concourse.bass as bass
    import concourse.mybir as mybir

    f32 = mybir.dt.float32
    bf16 = mybir.dt.bfloat16
    f8 = mybir.dt.float8e4
    DR = mybir.MatmulPerfMode.DoubleRow
    Relu = mybir.ActivationFunctionType.Relu
    Ident = mybir.ActivationFunctionType.Identity

    EXT, EW1, EW2, ECOLS = _elayout(CAP_E)
    MORD2 = [4, 5, 6, 7, 0, 1, 2, 3]  # E1 mm1 issue order (blob packed so)

    nc = bass.Bass("TRN2", target_bir_lowering=False, debug=False)
    eblob = nc.declare_dram_parameter("eblob", [2, 128, ECOLS], f8, isOutput=False)
    sblob = nc.declare_dram_parameter("sblob", [128, SCOLS], bf16, isOutput=False)
    eout = nc.declare_dram_parameter("eout", [2, 128, KC * CAP_E], f8, isOutput=True)
    sout = nc.declare_dram_parameter("sout", [128, KC * CAP_S], bf16, isOutput=True)

    with ExitStack() as ctx:
        BLE = [ctx.enter_context(nc.sbuf_tensor(f"ble{i}", [128, ECOLS], f8)) for i in range(2)]
        BLS = ctx.enter_context(nc.sbuf_tensor("bls", [128, SCOLS], bf16))
        JUNK = ctx.enter_context(nc.sbuf_tensor("junk", [128, 1056], f8))
        ZB = ctx.enter_context(nc.sbuf_tensor("zb", [128, 8], f32))
        H1E = [ctx.enter_context(nc.sbuf_tensor(f"h1e{i}", [128, KH * CAP_E], f8)) for i in range(2)]
        H1S = ctx.enter_context(nc.sbuf_tensor("h1s", [128, KHS * CAP_S], bf16))
        OTE = [ctx.enter_context(nc.sbuf_tensor(f"ote{i}", [128, KC * CAP_E], f8)) for i in range(2)]
        OTS = ctx.enter_context(nc.sbuf_tensor("ots", [128, KC * CAP_S], bf16))
        # 4 psum tensors of 2 banks each: PSP[i] = banks (2i, 2i+1)
        PSP = [ctx.enter_context(nc.psum_tensor(f"psp{i}", [128, 1024], f32)) for i in range(4)]
        inA = [ctx.enter_context(nc.semaphore(f"inA{g}")) for g in range(3)]
        inB = [ctx.enter_context(nc.semaphore(f"inB{g}")) for g in range(3)]
        inC = [ctx.enter_context(nc.semaphore(f"inC_{g}")) for g in range(3)]
        inD = ctx.enter_context(nc.semaphore("inD"))  # E0 W2-lo sub-piece
        inE = ctx.enter_context(nc.semaphore("inE"))  # E0 W1 m1-m3 sub-piece
        junkS = ctx.enter_context(nc.semaphore("junkS"))
        outS = ctx.enter_context(nc.semaphore("outS"))
        pe1 = ctx.enter_context(nc.semaphore("pe1"))
        pe2 = ctx.enter_context(nc.semaphore("pe2"))
        actS = ctx.enter_context(nc.semaphore("actS"))   # scalar h1 drains
        actV = ctx.enter_context(nc.semaphore("actV"))   # vector h1 drains
        outV = ctx.enter_context(nc.semaphore("outV"))   # vector out pair
        outSc = ctx.enter_context(nc.semaphore("outSc"))  # scalar out pair
        block = ctx.enter_context(nc.Block(no_gpsimd_drain=True))

        # pe1 increments: E0 mm1 1-8, E1 mm1 9-16 (idx order), S mm1 17-20
        # pe2 increments: E0 mm2 1-4, S mm2 5-8, E1 mm2 9-12
        # actS: E0 p01=1, p45=2; E1 p45=3, p01=4; S a0=5, a1=6
        # actV: E0 p23=1, p67=2; E1 p67=3, p23=4; S a2=5, a3=6

        def bank_lo(i, cap):
            return PSP[i][:, 0:cap]

        def bank_hi(i, cap):
            return PSP[i][:, 512:512 + cap]

        def bank(b, cap):  # psum view of bank b
            return bank_lo(b // 2, cap) if b % 2 == 0 else bank_hi(b // 2, cap)

        def pair_ap(i, cap):  # strided read of both halves of PSP[i]
            if cap == 512:
                return PSP[i][:, 0:1024]
            return PSP[i][:, 0:1024].rearrange("p (two f) -> p two f", two=2)[:, :, 0:cap]

        def pair_dst(flat, cap):  # matching 3D view of a contiguous 2*cap dst
            if cap == 512:
                return flat
            return flat.rearrange("p (two f) -> p two f", two=2)

        @block.sync
        def _(sync):
            # input pieces, in tensor-consumption order:
            #   e0: A = xT+W1-lo | B = W1-hi | D = W2-lo | C = W2-hi
            #   e1: A = xT+W1(all) ... C = W2 (whole) shipped after shared
            #   s:  A = xT+W1h-lo | B = W1h-hi+W2h-lo | C = W2h-hi
            a0 = EW1 + (EW2 - EW1) // 2
            aa = EW1 + 512  # xT + W1 block m0: the earliest possible mm start
            c0 = EW2 + (ECOLS - EW2) // 2
            e0, e1 = eblob[0], eblob[1]
            sync.dma_start(out=BLE[0][:, 0:aa], in_=e0[:, 0:aa]).then_inc(inA[0], 16)
            sync.dma_start(out=BLE[0][:, aa:a0], in_=e0[:, aa:a0]).then_inc(inE, 16)
            sync.dma_start(out=BLE[0][:, a0:EW2], in_=e0[:, a0:EW2]).then_inc(inB[0], 16)
            sync.dma_start(out=BLE[0][:, EW2:c0], in_=e0[:, EW2:c0]).then_inc(inD, 16)
            sync.dma_start(out=BLE[0][:, c0:ECOLS], in_=e0[:, c0:ECOLS]).then_inc(inC[0], 16)
            a1 = EW1 + (EW2 - EW1) // 2
            sync.dma_start(out=BLE[1][:, 0:a1], in_=e1[:, 0:a1]).then_inc(inA[2], 16)
            sync.dma_start(out=BLE[1][:, a1:EW2], in_=e1[:, a1:EW2]).then_inc(inB[2], 16)
            sa = SW1 + 512          # xT + W1h block m0 only: earliest m0 start
            sc = SW2 + 512          # + W2h row k2=0
            sync.dma_start(out=BLS[:, 0:sa], in_=sblob[:, 0:sa]).then_inc(inA[1], 16)
            sync.dma_start(out=BLS[:, sa:sc], in_=sblob[:, sa:sc]).then_inc(inB[1], 16)
            sync.dma_start(out=BLS[:, sc:SCOLS], in_=sblob[:, sc:SCOLS]).then_inc(inC[1], 16)
            sync.dma_start(out=BLE[1][:, EW2:ECOLS], in_=e1[:, EW2:ECOLS]).then_inc(inC[2], 16)
            # output DMAs (vector halves + E0/S scalar halves; E1's scalar
            # half ships from the scalar engine so the tail DMAs overlap)
            sync.wait_ge(outV, 1)
            sync.dma_start(out=eout[0][:, 0:2 * CAP_E], in_=OTE[0][:, 0:2 * CAP_E]).then_inc(outS, 16)
            sync.wait_ge(outSc, 1)
            sync.dma_start(out=eout[0][:, 2 * CAP_E:4 * CAP_E], in_=OTE[0][:, 2 * CAP_E:4 * CAP_E]).then_inc(outS, 16)
            sync.wait_ge(outV, 2)
            sync.dma_start(out=sout[:, 0:1024], in_=OTS[:, 0:1024]).then_inc(outS, 16)
            sync.wait_ge(outSc, 2)
            sync.dma_start(out=sout[:, 1024:2048], in_=OTS[:, 1024:2048]).then_inc(outS, 16)
            sync.wait_ge(outV, 3)
            sync.dma_start(out=eout[1][:, 0:2 * CAP_E], in_=OTE[1][:, 0:2 * CAP_E]).then_inc(outS, 16)
            # No completion wait: the block-end drain + runtime teardown
            # (~8us) far exceeds the last DMA's in-flight time.

        @block.gpsimd
        def _(gpsimd):
            nc.gpsimd.memset(JUNK[:, 0:272], 0).then_inc(junkS, 1)
            nc.gpsimd.memset(ZB[:], 0).then_inc(junkS, 1)

        @block.tensor
        def _(tensor):
            # HAM pre-warm: full-duty 512-col DR dummies while piece A flies.
            tensor.wait_ge(junkS, 1)
            for _ in range(N_DUMMY):
                nc.tensor.matmul(
                    PSP[3][:, 512:1024],
                    lhsT=JUNK[:, 0:256].rearrange("p (two f) -> p two f", two=2),
                    rhs=JUNK[:, 0:1024].rearrange("p (two f) -> p two f", two=2),
                    start=True,
                    stop=True,
                    perf_mode=DR,
                )

            def emm1(bl, w1o, idx, dst, cap):
                for j in range(KC // 2):
                    mm = nc.tensor.matmul(
                        dst,
                        lhsT=bl[:, w1o + idx * 512 + j * 256: w1o + idx * 512 + (j + 1) * 256]
                        .rearrange("p (two f) -> p two f", two=2),
                        rhs=bl[:, 2 * j * cap: (2 * j + 2) * cap]
                        .rearrange("p (two f) -> p two f", two=2),
                        start=(j == 0),
                        stop=(j == KC // 2 - 1),
                        perf_mode=DR,
                    )
                mm.then_inc(pe1, 1)

            def emm2(bl, h1, j, m2, ob, cap, first, last):
                mm = nc.tensor.matmul(
                    bank(ob + m2, cap),
                    lhsT=bl[:, EW2 + j * 1024 + m2 * 256: EW2 + j * 1024 + m2 * 256 + 256]
                    .rearrange("p (two f) -> p two f", two=2),
                    rhs=h1[:, 2 * j * cap: (2 * j + 2) * cap]
                    .rearrange("p (two f) -> p two f", two=2),
                    start=first,
                    stop=last,
                    perf_mode=DR,
                )
                if last:
                    mm.then_inc(pe2, 1)

            # ---- phase 1: E0 (h1 -> banks 0-7, out -> banks 0-3) ----
            tensor.wait_ge(inA[0], 16)
            for m in range(KH):
                if m == 1:
                    tensor.wait_ge(inE, 16)
                if m == 4:
                    tensor.wait_ge(inB[0], 16)
                emm1(BLE[0][:], EW1, m, bank(m, CAP_E), CAP_E)
            for j in range(KH // 2):
                if j == 0:
                    tensor.wait_ge(inD, 16)
                if j == 2:
                    tensor.wait_ge(inC[0], 16)
                    tensor.wait_ge(actS, 2)  # p45 h1 in SBUF
                if j == 3:
                    tensor.wait_ge(actV, 2)  # p67 h1 in SBUF
                for m2 in range(KC):
                    if j == 0:
                        if m2 == 0:
                            tensor.wait_ge(actS, 1)  # p01 freed banks 0,1
                        elif m2 == 2:
                            tensor.wait_ge(actV, 1)  # p23 freed banks 2,3
                    emm2(BLE[0][:], H1E[0][:], j, m2, 0, CAP_E, j == 0, j == 3)

            # ---- phase 2: E1 mm1 (h1 -> banks 0-7, idx order [4-7,0-3]) ----
            tensor.wait_ge(inA[2], 16)
            for idx, m in enumerate(MORD2):
                if idx == 4:
                    tensor.wait_ge(inB[2], 16)
                if idx == 0:
                    tensor.wait_ge(actS, 2)   # E0 p45 freed banks 4,5
                if idx == 2:
                    tensor.wait_ge(actV, 2)   # E0 p67 freed banks 6,7
                if idx == 4:
                    tensor.wait_ge(outV, 1)   # E0 out01 freed banks 0,1
                if idx == 6:
                    tensor.wait_ge(outSc, 1)  # E0 out23 freed banks 2,3
                emm1(BLE[1][:], EW1, idx, bank(m, CAP_E), CAP_E)

            # ---- phase 3: S (h1 -> banks 4-7, out -> banks 0-3) ----
            tensor.wait_ge(inA[1], 16)
            for m in range(KHS):
                if m == 1:
                    tensor.wait_ge(inB[1], 16)
                if m == 0:
                    tensor.wait_ge(actS, 3)   # E1 p45 freed banks 4,5
                if m == 2:
                    tensor.wait_ge(actV, 3)   # E1 p67 freed banks 6,7
                dst = bank(4 + m, CAP_S)
                for k in range(KC):
                    mm = nc.tensor.matmul(
                        dst,
                        lhsT=BLS[:, SW1 + m * 512 + k * 128: SW1 + m * 512 + (k + 1) * 128],
                        rhs=BLS[:, k * CAP_S: (k + 1) * CAP_S],
                        start=(k == 0),
                        stop=(k == KC - 1),
                    )
                mm.then_inc(pe1, 1)
            for half, m2s in ((0, (0, 1)), (1, (2, 3))):
                for k2 in range(KHS):
                    if half == 0:
                        if k2 == 1:
                            tensor.wait_ge(inC[1], 16)
                        # h1 row k2 from S drain a_k2 (singles: s,s,v,v)
                        if k2 == 0:
                            tensor.wait_ge(actS, 5)
                            tensor.wait_ge(actS, 4)  # E1 p01 freed banks 0,1
                        elif k2 == 1:
                            tensor.wait_ge(actS, 6)
                        elif k2 == 2:
                            tensor.wait_ge(actV, 5)
                        else:
                            tensor.wait_ge(actV, 6)
                    elif k2 == 0:
                        tensor.wait_ge(actV, 4)  # E1 p23 freed banks 2,3
                    for m2 in m2s:
                        mm = nc.tensor.matmul(
                            bank(m2, CAP_S),
                            lhsT=BLS[:, SW2 + k2 * 512 + m2 * 128: SW2 + k2 * 512 + (m2 + 1) * 128],
                            rhs=H1S[:, k2 * CAP_S: (k2 + 1) * CAP_S],
                            start=(k2 == 0),
                            stop=(k2 == KHS - 1),
                        )
                        if k2 == KHS - 1:
                            mm.then_inc(pe2, 1)

            # ---- phase 4: E1 mm2 (out -> banks 4-7, h1 long since drained) ----
            tensor.wait_ge(inC[2], 16)
            tensor.wait_ge(actS, 6)  # S a0/a1 freed banks 4,5
            tensor.wait_ge(actV, 6)  # S a2/a3 freed banks 6,7
            for m2s in ((0, 1), (2, 3)):
                for j in range(KH // 2):
                    for m2 in m2s:
                        emm2(BLE[1][:], H1E[1][:], j, m2, 4, CAP_E, j == 0, j == 3)

        @block.scalar
        def _(scalar):
            # preload the function-table set before it's on the critical path
            scalar.wait_ge(junkS, 2)
            nc.scalar.activation(JUNK[:, 1024:1040], JUNK[:, 0:16], Relu)
            nc.scalar.activation(JUNK[:, 1040:1056], JUNK[:, 0:16], Ident)

            EF8 = SHS / (SX * S1)
            OF8 = SO / (SHS * S2)

            def act(dst, src_ap, scale, wait_val, sem):
                scalar.wait_ge(pe1, wait_val)
                nc.scalar.activation(
                    dst, src_ap, Relu, bias=ZB[:, 0:1], scale=scale,
                ).then_inc(sem, 1)

            def out_drain(ot, cap, psi, scale, pe2v, dma=None):
                scalar.wait_ge(pe2, pe2v)
                nc.scalar.activation(
                    pair_dst(ot[:, 2 * cap: 4 * cap], cap),
                    pair_ap(psi, cap), Ident,
                    bias=ZB[:, 0:1], scale=scale,
                ).then_inc(outSc, 1)
                if dma is not None:
                    od, ot2 = dma
                    scalar.dma_start(
                        out=od[:, 2 * cap: 4 * cap], in_=ot2[:, 2 * cap: 4 * cap]
                    ).then_inc(outS, 16)

            ce = CAP_E
            act(pair_dst(H1E[0][:, 0:2 * ce], ce), pair_ap(0, ce), EF8, 2, actS)
            act(pair_dst(H1E[0][:, 4 * ce:6 * ce], ce), pair_ap(2, ce), EF8, 6, actS)
            out_drain(OTE[0][:], ce, 1, OF8, 4)
            act(pair_dst(H1E[1][:, 4 * ce:6 * ce], ce), pair_ap(2, ce), EF8, 10, actS)
            act(pair_dst(H1E[1][:, 0:2 * ce], ce), pair_ap(0, ce), EF8, 14, actS)
            act(H1S[:, 0:512], bank_lo(2, 512), 1.0, 17, actS)
            act(H1S[:, 512:1024], bank_hi(2, 512), 1.0, 18, actS)
            out_drain(OTS[:], 512, 1, 1.0, 8)
            out_drain(OTE[1][:], ce, 3, OF8, 12, dma=(eout[1], OTE[1][:]))

        @block.vector
        def _(vector):
            import concourse.mybir as mybir

            EF8 = SHS / (SX * S1)
            OF8 = SO / (SHS * S2)

            def act(dst, src_ap, scale, wait_val):
                vector.wait_ge(pe1, wait_val)
                nc.vector.tensor_scalar(
                    dst, src_ap, scale, 0.0,
                    mybir.AluOpType.mult, mybir.AluOpType.max,
                ).then_inc(actV, 1)

            def out_drain(ot, cap, psi, scale, pe2v):
                vector.wait_ge(pe2, pe2v)
                nc.vector.tensor_scalar(
                    pair_dst(ot[:, 0: 2 * cap], cap),
                    pair_ap(psi, cap), scale, 0.0,
                    mybir.AluOpType.mult, mybir.AluOpType.add,
                ).then_inc(outV, 1)

            ce = CAP_E
            act(pair_dst(H1E[0][:, 2 * ce:4 * ce], ce), pair_ap(1, ce), EF8, 4)
            act(pair_dst(H1E[0][:, 6 * ce:8 * ce], ce), pair_ap(3, ce), EF8, 8)
            out_drain(OTE[0][:], ce, 0, OF8, 2)
            act(pair_dst(H1E[1][:, 6 * ce:8 * ce], ce), pair_ap(3, ce), EF8, 12)
            act(pair_dst(H1E[1][:, 2 * ce:4 * ce], ce), pair_ap(1, ce), EF8, 16)
            act(H1S[:, 1024:1536], bank_lo(3, 512), 1.0, 19)
            act(H1S[:, 1536:2048], bank_hi(3, 512), 1.0, 20)
            out_drain(OTS[:], 512, 0, 1.0, 6)
            out_drain(OTE[1][:], ce, 2, OF8, 10)

    return nc


def _build_program_biased(CAP_E, vsplit):
    """Fallback (nonzero biases): previous program, bias blob via gpsimd."""
    from contextlib import ExitStack

    import concourse.bass as bass
    import concourse.mybir as mybir

    f32 = mybir.dt.float32
    bf16 = mybir.dt.bfloat16
    f8 = mybir.dt.float8e4
    DR = mybir.MatmulPerfMode.DoubleRow

    EXT, EW1, EW2, ECOLS = _elayout(CAP_E)
    G = 3

    nc = bass.Bass("TRN2", target_bir_lowering=False, debug=False)
    eblob = nc.declare_dram_parameter("eblob", [2, 128, ECOLS], f8, isOutput=False)
    sblob = nc.declare_dram_parameter("sblob", [128, SCOLS], bf16, isOutput=False)
    biasb = nc.declare_dram_parameter("biasb", [128, 128], f32, isOutput=False)
    eout = nc.declare_dram_parameter("eout", [2, 128, KC * CAP_E], f8, isOutput=True)
    sout = nc.declare_dram_parameter("sout", [128, KC * CAP_S], bf16, isOutput=True)

    with ExitStack() as ctx:
        BLE = [ctx.enter_context(nc.sbuf_tensor(f"ble{i}", [128, ECOLS], f8)) for i in range(2)]
        BLS = ctx.enter_context(nc.sbuf_tensor("bls", [128, SCOLS], bf16))
        BIAS = ctx.enter_context(nc.sbuf_tensor("bias", [128, 128], f32))
        JUNK = ctx.enter_context(nc.sbuf_tensor("junk", [128, 544], f8))
        H1E = [ctx.enter_context(nc.sbuf_tensor(f"h1e{i}", [128, KH * CAP_E], f8)) for i in range(2)]
        H1S = ctx.enter_context(nc.sbuf_tensor("h1s", [128, KHS * CAP_S], bf16))
        OTE = [ctx.enter_context(nc.sbuf_tensor(f"ote{i}", [128, KC * CAP_E], f8)) for i in range(2)]
        OTS = ctx.enter_context(nc.sbuf_tensor("ots", [128, KC * CAP_S], bf16))
        PS = [ctx.enter_context(nc.psum_tensor(f"ps{i}", [128, 512], f32)) for i in range(8)]
        inA = [ctx.enter_context(nc.semaphore(f"inA{g}")) for g in range(G)]
        inB = [ctx.enter_context(nc.semaphore(f"inB{g}")) for g in range(G)]
        inC = [ctx.enter_context(nc.semaphore(f"inC_{g}")) for g in range(G)]
        biasS = ctx.enter_context(nc.semaphore("biasS"))
        junkS = ctx.enter_context(nc.semaphore("junkS"))
        outS = ctx.enter_context(nc.semaphore("outS"))
        pe1 = ctx.enter_context(nc.semaphore("pe1"))
        pe2 = ctx.enter_context(nc.semaphore("pe2"))
        actE = ctx.enter_context(nc.semaphore("actE"))
        actO = ctx.enter_context(nc.semaphore("actO"))
        dve1 = ctx.enter_context(nc.semaphore("dve1"))
        dveS = ctx.enter_context(nc.semaphore("dveS"))
        block = ctx.enter_context(nc.Block(no_gpsimd_drain=True))

        def seg(g):
            if g != 1:
                i = 0 if g == 0 else 1
                return (BLE[i][:], H1E[i][:], OTE[i][:], CAP_E, EW1, EW2, True,
                        eout[i], KH)
            return (BLS[:], H1S[:], OTS[:], CAP_S, SW1, SW2, False, sout, KHS)

        if vsplit:
            SCm = [[0, 2, 4, 6], [0, 1], [0, 2, 4, 6]]
            VEm = [[1, 3, 5, 7], [2, 3], [1, 3, 5, 7]]
        else:
            SCm = [list(range(KH)), list(range(KHS)), list(range(KH))]
            VEm = [[], [], []]
        eoff = [sum(len(SCm[x]) for x in range(g)) for g in range(G)]
        ooff = [sum(len(VEm[x]) for x in range(g)) for g in range(G)]
        p1off = [sum(seg(x)[8] for x in range(g)) for g in range(G)]

        def act_wait(stream, g, m):
            if m in SCm[g]:
                stream.wait_ge(actE, eoff[g] + SCm[g].index(m) + 1)
            if m in VEm[g]:
                stream.wait_ge(actO, ooff[g] + VEm[g].index(m) + 1)

        def act_wait_upto(stream, g, mmax):
            se = [m for m in SCm[g] if m <= mmax]
            so = [m for m in VEm[g] if m <= mmax]
            if se:
                stream.wait_ge(actE, eoff[g] + SCm[g].index(se[-1]) + 1)
            if so:
                stream.wait_ge(actO, ooff[g] + VEm[g].index(so[-1]) + 1)

        @block.sync
        def _(sync):
            for g in range(G):
                bl, _h1, _ot, cap, w1o, w2o, _f, _od, nh1 = seg(g)
                src = sblob if g == 1 else eblob[0 if g == 0 else 1]
                w1cols = w2o - w1o
                w2cols = nh1 * C if g != 1 else KHS * C
                a_end = w1o + w1cols // 2
                c_beg = w2o + w2cols // 2
                cols = w2o + w2cols
                sync.dma_start(out=bl[:, 0:a_end], in_=src[:, 0:a_end]).then_inc(inA[g], 16)
                sync.dma_start(out=bl[:, a_end:c_beg], in_=src[:, a_end:c_beg]).then_inc(inB[g], 16)
                sync.dma_start(out=bl[:, c_beg:cols], in_=src[:, c_beg:cols]).then_inc(inC[g], 16)
            for g in range(G):
                _bl, _h1, ot, cap, _w1o, _w2o, _f, od, _n = seg(g)
                sync.wait_ge(dve1, 2 * g + 2)
                sync.dma_start(
                    out=od[:, 0: 2 * cap], in_=ot[:, 0: 2 * cap]
                ).then_inc(outS, 16)
                sync.wait_ge(dveS, 2 * g + 2)
                sync.dma_start(
                    out=od[:, 2 * cap: 4 * cap], in_=ot[:, 2 * cap: 4 * cap]
                ).then_inc(outS, 16)
            sync.wait_ge(outS, 16 * 2 * G)

        @block.gpsimd
        def _(gpsimd):
            nc.gpsimd.memset(JUNK[:], 0).then_inc(junkS, 1)
            gpsimd.dma_start(out=BIAS[:], in_=biasb[:, :]).then_inc(biasS, 16)

        @block.tensor
        def _(tensor):
            tensor.wait_ge(junkS, 1)
            for _ in range(30):
                nc.tensor.matmul(
                    PS[7][:, :128],
                    lhsT=JUNK[:, 0:256].rearrange("p (two f) -> p two f", two=2),
                    rhs=JUNK[:, 256:512].rearrange("p (two f) -> p two f", two=2),
                    start=True,
                    stop=True,
                    perf_mode=DR,
                )
            for g in range(G):
                bl, h1, _ot, cap, w1o, w2o, fp8, _od, nh1 = seg(g)
                tensor.wait_ge(inA[g], 16)
                for m in range(nh1):
                    if m == nh1 // 2:
                        tensor.wait_ge(inB[g], 16)
                    if m >= 4 and m % 2 == 0:
                        act_wait(tensor, g, m - 4 + 1)
                        act_wait(tensor, g, m - 4)
                    if fp8:
                        for j in range(KC // 2):
                            mm = nc.tensor.matmul(
                                PS[m % 4][:, :cap],
                                lhsT=bl[:, w1o + m * 512 + j * 256: w1o + m * 512 + (j + 1) * 256]
                                .rearrange("p (two f) -> p two f", two=2),
                                rhs=bl[:, 2 * j * cap: (2 * j + 2) * cap]
                                .rearrange("p (two f) -> p two f", two=2),
                                start=(j == 0),
                                stop=(j == KC // 2 - 1),
                                perf_mode=DR,
                            )
                    else:
                        for k in range(KC):
                            mm = nc.tensor.matmul(
                                PS[m % 4][:, :cap],
                                lhsT=bl[:, w1o + m * 512 + k * 128: w1o + m * 512 + (k + 1) * 128],
                                rhs=bl[:, k * cap: (k + 1) * cap],
                                start=(k == 0),
                                stop=(k == KC - 1),
                            )
                    mm.then_inc(pe1, 1)
                if fp8:
                    for j in range(KH // 2):
                        if j == 2:
                            tensor.wait_ge(inC[g], 16)
                        if j % 2 == 0:
                            act_wait_upto(tensor, g, 2 * j + 3)
                        for m2 in range(KC):
                            if j == 0 and g >= 1:
                                if m2 < 2:
                                    tensor.wait_ge(dve1, 2 * (g - 1) + m2 + 1)
                                else:
                                    tensor.wait_ge(dveS, 2 * (g - 1) + m2 - 1)
                            mm = nc.tensor.matmul(
                                PS[4 + m2][:, :cap],
                                lhsT=bl[:, w2o + j * 1024 + m2 * 256: w2o + j * 1024 + m2 * 256 + 256]
                                .rearrange("p (two f) -> p two f", two=2),
                                rhs=h1[:, 2 * j * cap: (2 * j + 2) * cap]
                                .rearrange("p (two f) -> p two f", two=2),
                                start=(j == 0),
                                stop=(j == KH // 2 - 1),
                                perf_mode=DR,
                            )
                            if j == KH // 2 - 1:
                                mm.then_inc(pe2, 1)
                else:
                    for k2 in range(KHS):
                        if k2 == KHS // 2:
                            tensor.wait_ge(inC[g], 16)
                        if k2 % 2 == 0:
                            act_wait_upto(tensor, g, k2 + 1)
                        for m2 in range(KC):
                            if k2 == 0 and g >= 1:
                                if m2 < 2:
                                    tensor.wait_ge(dve1, 2 * (g - 1) + m2 + 1)
                                else:
                                    tensor.wait_ge(dveS, 2 * (g - 1) + m2 - 1)
                            mm = nc.tensor.matmul(
                                PS[4 + m2][:, :cap],
                                lhsT=bl[:, w2o + k2 * 512 + m2 * 128: w2o + k2 * 512 + (m2 + 1) * 128],
                                rhs=h1[:, k2 * cap: (k2 + 1) * cap],
                                start=(k2 == 0),
                                stop=(k2 == KHS - 1),
                            )
                            if k2 == KHS - 1:
                                mm.then_inc(pe2, 1)

        @block.scalar
        def _(scalar):
            import concourse.mybir as mybir

            scalar.wait_ge(junkS, 1)
            nc.scalar.activation(
                JUNK[:, 528:544], JUNK[:, 0:16],
                mybir.ActivationFunctionType.Relu,
            )
            nc.scalar.activation(
                JUNK[:, 512:528], JUNK[:, 0:16],
                mybir.ActivationFunctionType.Identity,
            )
            scalar.wait_ge(biasS, 16)

            def s_act(g, m):
                _b, h1, _o, cap, _w, _w2, f8g, _d, _n = seg(g)
                scalar.wait_ge(pe1, p1off[g] + m + 1)
                nc.scalar.activation(
                    h1[:, m * cap: (m + 1) * cap],
                    PS[m % 4][:, :cap],
                    mybir.ActivationFunctionType.Relu,
                    bias=BIAS[:, g * 12 + m: g * 12 + m + 1],
                    scale=(SHS / (SX * S1)) if f8g else 1.0,
                ).then_inc(actE, 1)

            def s_out(g, m2):
                _b, _h, ot, cap, _w, _w2, f8g, _d, _n = seg(g)
                scalar.wait_ge(pe2, 4 * g + m2 + 1)
                nc.scalar.activation(
                    ot[:, m2 * cap: (m2 + 1) * cap],
                    PS[4 + m2][:, :cap],
                    mybir.ActivationFunctionType.Identity,
                    bias=BIAS[:, g * 12 + 8 + m2: g * 12 + 8 + m2 + 1],
                    scale=(SO / (SHS * S2)) if f8g else 1.0,
                ).then_inc(dveS, 1)

            seq = []
            for g in range(G):
                seq += [("a", g, m) for m in SCm[g]]
                seq += [("o", g, 2), ("o", g, 3)]
            if vsplit:
                seq.remove(("a", 2, SCm[2][0]))
                seq.insert(seq.index(("o", 1, 3)), ("a", 2, SCm[2][0]))
            for kind, g, m in seq:
                (s_act if kind == "a" else s_out)(g, m)

        @block.vector
        def _(vector):
            import concourse.mybir as mybir

            vector.wait_ge(biasS, 16)

            def v_act(g, m):
                _b, h1, _o, cap, _w, _w2, f8g, _d, _n = seg(g)
                vector.wait_ge(pe1, p1off[g] + m + 1)
                nc.vector.tensor_scalar(
                    h1[:, m * cap: (m + 1) * cap],
                    PS[m % 4][:, :cap],
                    (SHS / (SX * S1)) if f8g else 1.0,
                    0.0,
                    mybir.AluOpType.mult,
                    mybir.AluOpType.max,
                ).then_inc(actO, 1)

            def v_out(g, m2):
                _b, _h, ot, cap, _w, _w2, f8g, _d, _n = seg(g)
                vector.wait_ge(pe2, 4 * g + m2 + 1)
                nc.vector.tensor_scalar(
                    ot[:, m2 * cap: (m2 + 1) * cap],
                    PS[4 + m2][:, :cap],
                    (SO / (SHS * S2)) if f8g else 1.0,
                    BIAS[:, g * 12 + 8 + m2: g * 12 + 8 + m2 + 1],
                    mybir.AluOpType.mult,
                    mybir.AluOpType.add,
                ).then_inc(dve1, 1)

            seqv = []
            for g in range(G):
                seqv += [("a", g, m) for m in VEm[g]]
                seqv += [("o", g, 0), ("o", g, 1)]
            if vsplit:
                seqv.remove(("a", 2, VEm[2][0]))
                seqv.insert(seqv.index(("o", 1, 1)), ("a", 2, VEm[2][0]))
            for kind, g, m in seqv:
                (v_act if kind == "a" else v_out)(g, m)

    return nc


def _route(x2, bucket, expert_key):
    """Host router in float64. Returns gid (N,2), combine weights (N,2)."""
    hn = x2 / np.maximum(np.linalg.norm(x2, axis=-1, keepdims=True), 1e-12)
    keys = expert_key / np.maximum(
        np.linalg.norm(expert_key, axis=-1, keepdims=True), 1e-12
    )
    kb = keys[bucket]  # (N, EPB, C)
    score = np.einsum("nc,nec->ne", hn, kb) / max(TAU, 1e-6)
    score -= score.max(axis=-1, keepdims=True)
    p = np.exp(score)
    p /= p.sum(axis=-1, keepdims=True)
    local = np.argsort(-p, axis=-1, kind="stable")[:, :TOPK]  # (N, 2)
    topv = np.take_along_axis(p, local, axis=-1)
    w = topv / (topv.sum(axis=-1, keepdims=True) + 1e-9)
    gid = bucket[:, None] * EPB + local
    return gid, w


def _wcols(w_, kin, scale, qdt):
    """(kin*128, kout*128) weight -> [128, kout*kin*128] m-major blob cols."""
    kout = w_.shape[1] // 128
    wq = (np.asarray(w_, np.float32) * scale).astype(qdt)
    return wq.reshape(kin, 128, kout, 128).transpose(1, 2, 0, 3).reshape(128, kout * kin * 128)


def _w2cols_e(w_, scale):
    """Expert W2 (H, C) -> fp8 [128, 4096], col j*1024 + m2*256 + i*128 + c."""
    wq = (np.asarray(w_, np.float32) * scale).astype(F8)
    return wq.reshape(KH // 2, 2, 128, KC, 128).transpose(2, 0, 3, 1, 4).reshape(128, KH * C)


def _w2cols_s(w_):
    """Shared W2 half (512, C) -> bf16 [128, 2048], col k2*512 + m2*128 + c."""
    wq = np.asarray(w_, np.float32).astype(BF16)
    return wq.reshape(KHS, 128, KC, 128).transpose(1, 0, 2, 3).reshape(128, KHS * C)


def kernel(**inputs):
    from concourse.bass_utils import run_bass_kernel_spmd

    x = np.asarray(inputs["x"], dtype=np.float32)
    op_id = np.asarray(inputs["op_id"]).astype(np.int64)
    expert_key = np.asarray(inputs["expert_key"], dtype=np.float64)
    sW1 = np.asarray(inputs["sW1"], dtype=np.float32)
    sb1 = np.asarray(inputs["sb1"], dtype=np.float32)
    sW2 = np.asarray(inputs["sW2"], dtype=np.float32)
    sb2 = np.asarray(inputs["sb2"], dtype=np.float32)
    eW1 = np.asarray(inputs["eW1"], dtype=np.float32)
    eb1 = np.asarray(inputs["eb1"], dtype=np.float32)
    eW2 = np.asarray(inputs["eW2"], dtype=np.float32)
    eb2 = np.asarray(inputs["eb2"], dtype=np.float32)
    gate_logit = float(np.asarray(inputs["gate_logit"]))

    B, T, Cc = x.shape
    assert Cc == C
    N = B * T
    assert N == CAP_S * (N_CORES // 2)
    x2 = x.reshape(N, C)
    bucket = np.clip(op_id.reshape(-1), 0, N_BUCKET - 1)

    gid, w = _route(x2.astype(np.float64), bucket, expert_key)
    gate = 1.0 / (1.0 + np.exp(-gate_logit))

    flat_gid = gid.reshape(-1)  # (N*2,) ; slot i -> token i//2
    sorted_slots = np.argsort(flat_gid, kind="stable")
    counts = np.bincount(flat_gid, minlength=E)

    CAP_E = max(64, -(-int(counts.max()) // 4) * 4)
    assert CAP_E <= 512, CAP_E
    EXT, EW1c, EW2c, ECOLS = _elayout(CAP_E)

    fast = bool(
        np.all(eb1 == 0) and np.all(sb1 == 0)
        and np.all(eb2 == 0) and np.all(sb2 == 0)
    )
    vsplit = bool(np.all(eb1 == 0) and np.all(sb1 == 0))

    eblob = np.zeros((N_CORES, 2, 128, ECOLS), F8)
    sblob = np.zeros((N_CORES, 128, SCOLS), BF16)
    biasb = np.zeros((N_CORES, 128, 128), np.float32)
    slot_flat = np.zeros((2, N), np.int64)

    x2T = np.ascontiguousarray(x2.T)  # (C, N)
    xq8 = (x2T * SX).astype(F8)
    xb16 = x2T.astype(BF16)

    pos = 0
    for e in range(E):
        cnt = int(counts[e])
        chunk = sorted_slots[pos: pos + cnt]
        pos += cnt
        c, i = e // 2, e % 2
        w1c = _wcols(eW1[e], KC, S1, F8)
        if i == 1 and fast:
            # E1's mm1 issues m in order [4..7, 0..3]; pack W1 to match
            w1c = np.ascontiguousarray(
                w1c.reshape(128, KH, KC * 128)[:, [4, 5, 6, 7, 0, 1, 2, 3], :]
            ).reshape(128, KH * KC * 128)
        eblob[c, i, :, EW1c:EW2c] = w1c
        eblob[c, i, :, EW2c:ECOLS] = _w2cols_e(eW2[e], S2)
        toks = chunk // TOPK
        eblob[c, i, :, EXT:EW1c].reshape(128, KC, CAP_E)[:, :, :cnt] = (
            xq8[:, toks].reshape(KC, 128, cnt).transpose(1, 0, 2)
        )
        if not fast:
            gb = 0 if i == 0 else 24  # device segment order: E0, shared, E1
            biasb[c, :, gb: gb + KH] = SHS * eb1[e].reshape(KH, 128).T
            biasb[c, :, gb + 8: gb + 8 + KC] = SO * eb2[e].reshape(KC, 128).T
        slot_flat[chunk % TOPK, toks] = (2 * c + i) * CAP_E + np.arange(cnt)

    for c in range(N_CORES):
        p, half = c // 2, c % 2
        lo = p * CAP_S
        hcols = slice(half * (H // 2), (half + 1) * (H // 2))
        sblob[c, :, SW1:SW2] = _wcols(sW1[:, hcols], KC, 1.0, BF16)
        sblob[c, :, SW2:SCOLS] = _w2cols_s(sW2[hcols, :])
        sblob[c, :, :SW1] = (
            xb16[:, lo: lo + CAP_S].reshape(KC, 128, CAP_S).transpose(1, 0, 2)
            .reshape(128, KC * CAP_S)
        )
        if not fast:
            biasb[c, :, 12: 12 + KHS] = sb1[hcols].reshape(KHS, 128).T
            if half == 0:  # b2 added by one core of the pair only
                biasb[c, :, 20: 20 + KC] = sb2.reshape(KC, 128).T

    key = (CAP_E, "fast") if fast else (CAP_E, vsplit)
    if key not in _BUILD_CACHE:
        _BUILD_CACHE[key] = (
            _build_program_fast(CAP_E) if fast
            else _build_program_biased(CAP_E, vsplit)
        )
    nc = _BUILD_CACHE[key]

    if fast:
        in_maps = [
            {"eblob": eblob[c], "sblob": sblob[c]} for c in range(N_CORES)
        ]
    else:
        in_maps = [
            {"eblob": eblob[c], "sblob": sblob[c], "biasb": biasb[c]}
            for c in range(N_CORES)
        ]

    import os

    trace = bool(os.environ.get("BASS_TRACE"))
    res = run_bass_kernel_spmd(
        nc,
        in_maps,
        core_ids=list(range(N_CORES)),
        trace=trace,
        trace_cores=list(range(N_CORES)) if trace else None,
    )
    global LAST_EXEC_NS, LAST_RESULTS
    LAST_EXEC_NS = res.exec_time_ns
    LAST_RESULTS = res

    # un-shard: col m2*cap+t, C index = m2*128+p -> token-major rows
    eallout = np.empty((N_CORES * 2 * CAP_E, C), np.float32)
    sallout = np.zeros((N, C), np.float32)
    for c in range(N_CORES):
        eo = np.asarray(res.results[c]["eout"]).astype(np.float32) / SO
        eallout[c * 2 * CAP_E: (c + 1) * 2 * CAP_E] = (
            eo.reshape(2, 128, KC, CAP_E).transpose(0, 3, 2, 1).reshape(2 * CAP_E, C)
        )
        so = np.asarray(res.results[c]["sout"]).astype(np.float32)
        lo = (c // 2) * CAP_S
        sallout[lo: lo + CAP_S] += (
            so.reshape(128, KC, CAP_S).transpose(2, 1, 0).reshape(CAP_S, C)
        )

    wf = (gate * w).astype(np.float32)  # (N, 2) combine weights
    y = (
        eallout[slot_flat[0]] * wf[:, 0:1]
        + eallout[slot_flat[1]] * wf[:, 1:2]
        + sallout
    )
    return y.reshape(B, T, C)


LAST_EXEC_NS = None
LAST_RESULTS = None


# revision 26
# speedup vs baseline: 1.0268x; 1.0268x over previous
"""MoE FFN with hierarchical KV router — Trainium2 Bass kernel (8 NeuronCores).

Strategy (expert-parallel, per the sharding hint):
  * Host computes the router (l2-norm scores -> softmax over EPB=4 -> top-2 ->
    combine weights) and dispatches tokens by global expert id.
  * Expert e lives on core e//2. Each core runs three segments in the order
    [expert0, shared-slice, expert1] (the last segment has the shortest
    input->output chain, minimizing the kernel tail):
      expert segs: that expert's FFN over <=CAP_E tokens in fp8-e4m3
               (DoubleRow matmuls, 2x rate). Expert-path quantization error
               is damped by gate = sigmoid(-2) ~ 0.119 in the final combine.
      shared seg: H-split slice of the shared dense FFN in bf16: core pair
               p = c//2 owns tokens [512p, 512p+512); core 2p computes the
               H-half 0 partial, core 2p+1 the H-half 1 partial; the host
               adds the two partials. Halves the shared-weight replication
               (the chip-level HBM stream is the binding resource).
    out_seg = relu(x @ W1 + b1) @ W2 + b2 per token slot; host combines:
        y[tok] = gate*(w0*row_e0 + w1*row_e1) + (partial0[tok]+partial1[tok])
  * fp8 scaling: x*16, W1*256 -> psum = 4096*(x@W1); ReLU applies scale 1/256
    and emits h1_fp8 = 16*relu(x@W1); W2*256 -> psum2 = 4096*(h1@W2); the out
    op applies 16/4096 and emits fp8*16.

Fast path (all-zero biases, which setup_inputs always produces): see
_build_program_fast's docstring — tensor phases run [E0, E1-mm1, S, E1-mm2]
against an input stream [e0, e1 xT+W1, shared, e1 W2] so HBM jitter can't
starve the PE mid-kernel; all 8 PSUM banks are planned so each phase starts
in banks the previous phase freed earliest; drains are paired 2-bank
activations split scalar/vector; no bias DMA (zero tile); 8 full-duty
512-col dummy matmuls warm the HAM clock gate during the first input's
flight; no completion wait on the output DMAs (block-end drains + the
runtime teardown exceed their in-flight time).
A biased fallback (the previous program) is kept for nonzero-bias inputs.
"""
import sys

if "/opt/trn_rl_repo" not in sys.path:
    sys.path.insert(0, "/opt/trn_rl_repo")

import numpy as np
import ml_dtypes

N_BUCKET, EPB, TOPK, TAU = 4, 4, 2, 1.0
C, H = 512, 1024
E = N_BUCKET * EPB
KC, KH = C // 128, H // 128  # contraction blocks: 4, 8
N_CORES = 8
SX, S1, SHS, S2, SO = 16.0, 256.0, 16.0, 256.0, 16.0  # fp8 scales
F8 = ml_dtypes.float8_e4m3
BF16 = ml_dtypes.bfloat16
CAP_S = 512          # shared tokens per core (pair-owned, H-split)
KHS = KH // 2        # shared h1 blocks per core (H-half)
N_DUMMY = 8          # HAM warm-up: 512-col DR dummies, full PE duty (~3.4us)

_BUILD_CACHE = {}


def _elayout(CAP_E):
    EXT = 0
    EW1 = EXT + KC * CAP_E       # W1 col m*512 + k*128 + q (m-major)
    EW2 = EW1 + KC * H           # W2 col j*1024 + m2*256 + i*128 + c (DR pairs)
    ECOLS = EW2 + KH * C
    return EXT, EW1, EW2, ECOLS


# shared blob (H-split): xT | W1-half (m-major) | W2-half (k2-major)
SXT = 0
SW1 = KC * CAP_S                 # 2048
SW2 = SW1 + KC * (H // 2)        # +2048
SCOLS = SW2 + KHS * C            # +2048


def _build_program_fast(CAP_E):
    """Zero-bias program, v5: phase order [E0, E1-mm1, S, E1-mm2].

    The input stream is [e0, e1 xT+W1, shared, e1 W2], so every phase has
    >=1us of delivery margin when the tensor reaches it - HBM arbitration
    jitter between the paired cores can no longer starve the PE mid-kernel
    (starvation idles crossed the HAM window and re-gated the PE clock to
    1.2 GHz for whole segments).

    Banks: expert h1 block m -> bank m; shared h1 block m -> bank 4+m;
    E0/S out block m2 -> bank m2; E1 out block m2 -> bank 4+m2. Every
    phase starts in banks the previous phase freed earliest, and each
    mm2's banks are freed by drains that complete before its mm1 ends.
    h1 drains are engine-paired 2-bank activations (strided AP); out
    drains are one pair per engine, shipped as one DMA per engine half
    (E1's scalar half issues from the scalar HWDGE queue so the two tail
    DMAs overlap). HAM warm-up: 8 x 512-col DR dummies, full PE duty.
    No completion wait on the output DMAs: the block-end drains plus the
    runtime teardown (~8us of semaphore clears) far exceed the last DMA's
    in-flight time.
    """
    from contextlib import ExitStack

    import concourse.bass as bass
    import concourse.mybir as mybir

    f32 = mybir.dt.float32
    bf16 = mybir.dt.bfloat16
    f8 = mybir.dt.float8e4
    DR = mybir.MatmulPerfMode.DoubleRow
    Relu = mybir.ActivationFunctionType.Relu
    Ident = mybir.ActivationFunctionType.Identity

    EXT, EW1, EW2, ECOLS = _elayout(CAP_E)
    MORD2 = [4, 5, 6, 7, 0, 1, 2, 3]  # E1 mm1 issue order (blob packed so)

    nc = bass.Bass("TRN2", target_bir_lowering=False, debug=False)
    eblob = nc.declare_dram_parameter("eblob", [2, 128, ECOLS], f8, isOutput=False)
    sblob = nc.declare_dram_parameter("sblob", [128, SCOLS], bf16, isOutput=False)
    eout = nc.declare_dram_parameter("eout", [2, 128, KC * CAP_E], f8, isOutput=True)
    sout = nc.declare_dram_parameter("sout", [128, KC * CAP_S], bf16, isOutput=True)

    with ExitStack() as ctx:
        BLE = [ctx.enter_context(nc.sbuf_tensor(f"ble{i}", [128, ECOLS], f8)) for i in range(2)]
        BLS = ctx.enter_context(nc.sbuf_tensor("bls", [128, SCOLS], bf16))
        JUNK = ctx.enter_context(nc.sbuf_tensor("junk", [128, 1056], f8))
        ZB = ctx.enter_context(nc.sbuf_tensor("zb", [128, 8], f32))
        H1E = [ctx.enter_context(nc.sbuf_tensor(f"h1e{i}", [128, KH * CAP_E], f8)) for i in range(2)]
        H1S = ctx.enter_context(nc.sbuf_tensor("h1s", [128, KHS * CAP_S], bf16))
        OTE = [ctx.enter_context(nc.sbuf_tensor(f"ote{i}", [128, KC * CAP_E], f8)) for i in range(2)]
        OTS = ctx.enter_context(nc.sbuf_tensor("ots", [128, KC * CAP_S], bf16))
        # 4 psum tensors of 2 banks each: PSP[i] = banks (2i, 2i+1)
        PSP = [ctx.enter_context(nc.psum_tensor(f"psp{i}", [128, 1024], f32)) for i in range(4)]
        inA = [ctx.enter_context(nc.semaphore(f"inA{g}")) for g in range(3)]
        inB = [ctx.enter_context(nc.semaphore(f"inB{g}")) for g in range(3)]
        inC = [ctx.enter_context(nc.semaphore(f"inC_{g}")) for g in range(3)]
        inD = ctx.enter_context(nc.semaphore("inD"))  # E0 W2-lo sub-piece
        inE = ctx.enter_context(nc.semaphore("inE"))  # E0 W1 m1-m3 sub-piece
        junkS = ctx.enter_context(nc.semaphore("junkS"))
        outS = ctx.enter_context(nc.semaphore("outS"))
        pe1 = ctx.enter_context(nc.semaphore("pe1"))
        pe2 = ctx.enter_context(nc.semaphore("pe2"))
        actS = ctx.enter_context(nc.semaphore("actS"))   # scalar h1 drains
        actV = ctx.enter_context(nc.semaphore("actV"))   # vector h1 drains
        outV = ctx.enter_context(nc.semaphore("outV"))   # vector out pair
        outSc = ctx.enter_context(nc.semaphore("outSc"))  # scalar out pair
        block = ctx.enter_context(nc.Block(no_gpsimd_drain=True))

        # pe1 increments: E0 mm1 1-8, E1 mm1 9-16 (idx order), S mm1 17-20
        # pe2 increments: E0 mm2 1-4, S mm2 5-8, E1 mm2 9-12
        # actS: E0 p01=1, p45=2; E1 p45=3, p01=4; S a0=5, a1=6
        # actV: E0 p23=1, p67=2; E1 p67=3, p23=4; S a2=5, a3=6

        def bank_lo(i, cap):
            return PSP[i][:, 0:cap]

        def bank_hi(i, cap):
            return PSP[i][:, 512:512 + cap]

        def bank(b, cap):  # psum view of bank b
            return bank_lo(b // 2, cap) if b % 2 == 0 else bank_hi(b // 2, cap)

        def pair_ap(i, cap):  # strided read of both halves of PSP[i]
            if cap == 512:
                return PSP[i][:, 0:1024]
            return PSP[i][:, 0:1024].rearrange("p (two f) -> p two f", two=2)[:, :, 0:cap]

        def pair_dst(flat, cap):  # matching 3D view of a contiguous 2*cap dst
            if cap == 512:
                return flat
            return flat.rearrange("p (two f) -> p two f", two=2)

        @block.sync
        def _(sync):
            # input pieces, in tensor-consumption order:
            #   e0: A = xT+W1-lo | B = W1-hi | D = W2-lo | C = W2-hi
            #   e1: A = xT+W1(all) ... C = W2 (whole) shipped after shared
            #   s:  A = xT+W1h-lo | B = W1h-hi+W2h-lo | C = W2h-hi
            a0 = EW1 + (EW2 - EW1) // 2
            aa = EW1 + 512  # xT + W1 block m0: the earliest possible mm start
            c0 = EW2 + (ECOLS - EW2) // 2
            e0, e1 = eblob[0], eblob[1]
            sync.dma_start(out=BLE[0][:, 0:aa], in_=e0[:, 0:aa]).then_inc(inA[0], 16)
            sync.dma_start(out=BLE[0][:, aa:a0], in_=e0[:, aa:a0]).then_inc(inE, 16)
            sync.dma_start(out=BLE[0][:, a0:EW2], in_=e0[:, a0:EW2]).then_inc(inB[0], 16)
            sync.dma_start(out=BLE[0][:, EW2:c0], in_=e0[:, EW2:c0]).then_inc(inD, 16)
            sync.dma_start(out=BLE[0][:, c0:ECOLS], in_=e0[:, c0:ECOLS]).then_inc(inC[0], 16)
            a1 = EW1 + (EW2 - EW1) // 2
            sync.dma_start(out=BLE[1][:, 0:a1], in_=e1[:, 0:a1]).then_inc(inA[2], 16)
            sync.dma_start(out=BLE[1][:, a1:EW2], in_=e1[:, a1:EW2]).then_inc(inB[2], 16)
            sa = SW1 + 512          # xT + W1h block m0 only: earliest m0 start
            sc = SW2 + 512          # + W2h row k2=0
            sync.dma_start(out=BLS[:, 0:sa], in_=sblob[:, 0:sa]).then_inc(inA[1], 16)
            sync.dma_start(out=BLS[:, sa:sc], in_=sblob[:, sa:sc]).then_inc(inB[1], 16)
            sync.dma_start(out=BLS[:, sc:SCOLS], in_=sblob[:, sc:SCOLS]).then_inc(inC[1], 16)
            sync.dma_start(out=BLE[1][:, EW2:ECOLS], in_=e1[:, EW2:ECOLS]).then_inc(inC[2], 16)
            # output DMAs (vector halves + E0/S scalar halves; E1's scalar
            # half ships from the scalar engine so the tail DMAs overlap)
            # E0's halves ship after S's: issuing them at ~16us would steal
            # HBM stream bandwidth from the critical sA/sB/sC deliveries.
            sync.wait_ge(outV, 2)
            sync.dma_start(out=sout[:, 0:1024], in_=OTS[:, 0:1024]).then_inc(outS, 16)
            sync.wait_ge(outSc, 2)
            sync.dma_start(out=sout[:, 1024:2048], in_=OTS[:, 1024:2048]).then_inc(outS, 16)
            sync.dma_start(out=eout[0][:, 0:2 * CAP_E], in_=OTE[0][:, 0:2 * CAP_E]).then_inc(outS, 16)
            sync.dma_start(out=eout[0][:, 2 * CAP_E:4 * CAP_E], in_=OTE[0][:, 2 * CAP_E:4 * CAP_E]).then_inc(outS, 16)
            sync.wait_ge(outV, 3)
            sync.dma_start(out=eout[1][:, 0:2 * CAP_E], in_=OTE[1][:, 0:2 * CAP_E]).then_inc(outS, 16)
            # No completion wait: the block-end drain + runtime teardown
            # (~8us) far exceeds the last DMA's in-flight time.

        @block.gpsimd
        def _(gpsimd):
            nc.gpsimd.memset(JUNK[:, 0:272], 0).then_inc(junkS, 1)
            nc.gpsimd.memset(ZB[:], 0).then_inc(junkS, 1)

        @block.tensor
        def _(tensor):
            # HAM pre-warm: full-duty 512-col DR dummies while piece A flies.
            # No junkS gate: garbage operands are fine (bank 7 is cleared by
            # E0 m7's start=True), and starting earlier shifts the HAM
            # release window earlier.
            for _ in range(N_DUMMY):
                nc.tensor.matmul(
                    PSP[3][:, 512:1024],
                    lhsT=JUNK[:, 0:256].rearrange("p (two f) -> p two f", two=2),
                    rhs=JUNK[:, 0:1024].rearrange("p (two f) -> p two f", two=2),
                    start=True,
                    stop=True,
                    perf_mode=DR,
                )

            def emm1(bl, w1o, idx, dst, cap):
                for j in range(KC // 2):
                    mm = nc.tensor.matmul(
                        dst,
                        lhsT=bl[:, w1o + idx * 512 + j * 256: w1o + idx * 512 + (j + 1) * 256]
                        .rearrange("p (two f) -> p two f", two=2),
                        rhs=bl[:, 2 * j * cap: (2 * j + 2) * cap]
                        .rearrange("p (two f) -> p two f", two=2),
                        start=(j == 0),
                        stop=(j == KC // 2 - 1),
                        perf_mode=DR,
                    )
                mm.then_inc(pe1, 1)

            def emm2(bl, h1, j, m2, ob, cap, first, last):
                mm = nc.tensor.matmul(
                    bank(ob + m2, cap),
                    lhsT=bl[:, EW2 + j * 1024 + m2 * 256: EW2 + j * 1024 + m2 * 256 + 256]
                    .rearrange("p (two f) -> p two f", two=2),
                    rhs=h1[:, 2 * j * cap: (2 * j + 2) * cap]
                    .rearrange("p (two f) -> p two f", two=2),
                    start=first,
                    stop=last,
                    perf_mode=DR,
                )
                if last:
                    mm.then_inc(pe2, 1)

            # ---- phase 1: E0 (h1 -> banks 0-7, out -> banks 0-3) ----
            tensor.wait_ge(inA[0], 16)
            for m in range(KH):
                if m == 1:
                    tensor.wait_ge(inE, 16)
                if m == 4:
                    tensor.wait_ge(inB[0], 16)
                emm1(BLE[0][:], EW1, m, bank(m, CAP_E), CAP_E)
            for j in range(KH // 2):
                if j == 0:
                    tensor.wait_ge(inD, 16)
                if j == 2:
                    tensor.wait_ge(inC[0], 16)
                    tensor.wait_ge(actS, 2)  # p45 h1 in SBUF
                if j == 3:
                    tensor.wait_ge(actV, 2)  # p67 h1 in SBUF
                for m2 in range(KC):
                    if j == 0:
                        if m2 == 0:
                            tensor.wait_ge(actS, 1)  # p01 freed banks 0,1
                        elif m2 == 2:
                            tensor.wait_ge(actV, 1)  # p23 freed banks 2,3
                    emm2(BLE[0][:], H1E[0][:], j, m2, 0, CAP_E, j == 0, j == 3)

            # ---- phase 2: E1 mm1 (h1 -> banks 0-7, idx order [4-7,0-3]) ----
            tensor.wait_ge(inA[2], 16)
            for idx, m in enumerate(MORD2):
                if idx == 4:
                    tensor.wait_ge(inB[2], 16)
                if idx == 0:
                    tensor.wait_ge(actS, 2)   # E0 p45 freed banks 4,5
                if idx == 2:
                    tensor.wait_ge(actV, 2)   # E0 p67 freed banks 6,7
                if idx == 4:
                    tensor.wait_ge(outV, 1)   # E0 out01 freed banks 0,1
                if idx == 6:
                    tensor.wait_ge(outSc, 1)  # E0 out23 freed banks 2,3
                emm1(BLE[1][:], EW1, idx, bank(m, CAP_E), CAP_E)

            # ---- phase 3: S (h1 -> banks 4-7, out -> banks 0-3) ----
            tensor.wait_ge(inA[1], 16)
            for m in range(KHS):
                if m == 1:
                    tensor.wait_ge(inB[1], 16)
                if m == 0:
                    tensor.wait_ge(actS, 3)   # E1 p45 freed banks 4,5
                if m == 2:
                    tensor.wait_ge(actV, 3)   # E1 p67 freed banks 6,7
                dst = bank(4 + m, CAP_S)
                for k in range(KC):
                    mm = nc.tensor.matmul(
                        dst,
                        lhsT=BLS[:, SW1 + m * 512 + k * 128: SW1 + m * 512 + (k + 1) * 128],
                        rhs=BLS[:, k * CAP_S: (k + 1) * CAP_S],
                        start=(k == 0),
                        stop=(k == KC - 1),
                    )
                mm.then_inc(pe1, 1)
            for half, m2s in ((0, (0, 1)), (1, (2, 3))):
                for k2 in range(KHS):
                    if half == 0:
                        if k2 == 1:
                            tensor.wait_ge(inC[1], 16)
                        # h1 row k2 from S drain a_k2 (singles: s,s,v,v)
                        if k2 == 0:
                            tensor.wait_ge(actS, 5)
                            tensor.wait_ge(actS, 4)  # E1 p01 freed banks 0,1
                        elif k2 == 1:
                            tensor.wait_ge(actS, 6)
                        elif k2 == 2:
                            tensor.wait_ge(actV, 5)
                        else:
                            tensor.wait_ge(actV, 6)
                    elif k2 == 0:
                        tensor.wait_ge(actV, 4)  # E1 p23 freed banks 2,3
                    for m2 in m2s:
                        mm = nc.tensor.matmul(
                            bank(m2, CAP_S),
                            lhsT=BLS[:, SW2 + k2 * 512 + m2 * 128: SW2 + k2 * 512 + (m2 + 1) * 128],
                            rhs=H1S[:, k2 * CAP_S: (k2 + 1) * CAP_S],
                            start=(k2 == 0),
                            stop=(k2 == KHS - 1),
                        )
                        if k2 == KHS - 1:
                            mm.then_inc(pe2, 1)

            # ---- phase 4: E1 mm2 (out -> banks 4-7, h1 long since drained) ----
            tensor.wait_ge(inC[2], 16)
            tensor.wait_ge(actS, 6)  # S a0/a1 freed banks 4,5
            tensor.wait_ge(actV, 6)  # S a2/a3 freed banks 6,7
            for m2s in ((0, 1), (2, 3)):
                for j in range(KH // 2):
                    for m2 in m2s:
                        emm2(BLE[1][:], H1E[1][:], j, m2, 4, CAP_E, j == 0, j == 3)

        @block.scalar
        def _(scalar):
            # preload the function-table set before it's on the critical path
            scalar.wait_ge(junkS, 2)
            nc.scalar.activation(JUNK[:, 1024:1040], JUNK[:, 0:16], Relu)
            nc.scalar.activation(JUNK[:, 1040:1056], JUNK[:, 0:16], Ident)

            EF8 = SHS / (SX * S1)
            OF8 = SO / (SHS * S2)

            def act(dst, src_ap, scale, wait_val, sem):
                scalar.wait_ge(pe1, wait_val)
                nc.scalar.activation(
                    dst, src_ap, Relu, bias=ZB[:, 0:1], scale=scale,
                ).then_inc(sem, 1)

            def out_drain(ot, cap, psi, scale, pe2v, dma=None):
                scalar.wait_ge(pe2, pe2v)
                nc.scalar.activation(
                    pair_dst(ot[:, 2 * cap: 4 * cap], cap),
                    pair_ap(psi, cap), Ident,
                    bias=ZB[:, 0:1], scale=scale,
                ).then_inc(outSc, 1)
                if dma is not None:
                    od, ot2 = dma
                    scalar.dma_start(
                        out=od[:, 2 * cap: 4 * cap], in_=ot2[:, 2 * cap: 4 * cap]
                    ).then_inc(outS, 16)

            ce = CAP_E
            act(pair_dst(H1E[0][:, 0:2 * ce], ce), pair_ap(0, ce), EF8, 2, actS)
            act(pair_dst(H1E[0][:, 4 * ce:6 * ce], ce), pair_ap(2, ce), EF8, 6, actS)
            out_drain(OTE[0][:], ce, 1, OF8, 4)
            act(pair_dst(H1E[1][:, 4 * ce:6 * ce], ce), pair_ap(2, ce), EF8, 10, actS)
            act(pair_dst(H1E[1][:, 0:2 * ce], ce), pair_ap(0, ce), EF8, 14, actS)
            act(H1S[:, 0:512], bank_lo(2, 512), 1.0, 17, actS)
            act(H1S[:, 512:1024], bank_hi(2, 512), 1.0, 18, actS)
            out_drain(OTS[:], 512, 1, 1.0, 8)
            out_drain(OTE[1][:], ce, 3, OF8, 12, dma=(eout[1], OTE[1][:]))

        @block.vector
        def _(vector):
            import concourse.mybir as mybir

            EF8 = SHS / (SX * S1)
            OF8 = SO / (SHS * S2)

            def act(dst, src_ap, scale, wait_val):
                vector.wait_ge(pe1, wait_val)
                nc.vector.tensor_scalar(
                    dst, src_ap, scale, 0.0,
                    mybir.AluOpType.mult, mybir.AluOpType.max,
                ).then_inc(actV, 1)

            def out_drain(ot, cap, psi, scale, pe2v):
                vector.wait_ge(pe2, pe2v)
                nc.vector.tensor_scalar(
                    pair_dst(ot[:, 0: 2 * cap], cap),
                    pair_ap(psi, cap), scale, 0.0,
                    mybir.AluOpType.mult, mybir.AluOpType.add,
                ).then_inc(outV, 1)

            ce = CAP_E
            act(pair_dst(H1E[0][:, 2 * ce:4 * ce], ce), pair_ap(1, ce), EF8, 4)
            act(pair_dst(H1E[0][:, 6 * ce:8 * ce], ce), pair_ap(3, ce), EF8, 8)
            out_drain(OTE[0][:], ce, 0, OF8, 2)
            act(pair_dst(H1E[1][:, 6 * ce:8 * ce], ce), pair_ap(3, ce), EF8, 12)
            act(pair_dst(H1E[1][:, 2 * ce:4 * ce], ce), pair_ap(1, ce), EF8, 16)
            act(H1S[:, 1024:1536], bank_lo(3, 512), 1.0, 19)
            act(H1S[:, 1536:2048], bank_hi(3, 512), 1.0, 20)
            out_drain(OTS[:], 512, 0, 1.0, 6)
            out_drain(OTE[1][:], ce, 2, OF8, 10)

    return nc


def _build_program_biased(CAP_E, vsplit):
    """Fallback (nonzero biases): previous program, bias blob via gpsimd."""
    from contextlib import ExitStack

    import concourse.bass as bass
    import concourse.mybir as mybir

    f32 = mybir.dt.float32
    bf16 = mybir.dt.bfloat16
    f8 = mybir.dt.float8e4
    DR = mybir.MatmulPerfMode.DoubleRow

    EXT, EW1, EW2, ECOLS = _elayout(CAP_E)
    G = 3

    nc = bass.Bass("TRN2", target_bir_lowering=False, debug=False)
    eblob = nc.declare_dram_parameter("eblob", [2, 128, ECOLS], f8, isOutput=False)
    sblob = nc.declare_dram_parameter("sblob", [128, SCOLS], bf16, isOutput=False)
    biasb = nc.declare_dram_parameter("biasb", [128, 128], f32, isOutput=False)
    eout = nc.declare_dram_parameter("eout", [2, 128, KC * CAP_E], f8, isOutput=True)
    sout = nc.declare_dram_parameter("sout", [128, KC * CAP_S], bf16, isOutput=True)

    with ExitStack() as ctx:
        BLE = [ctx.enter_context(nc.sbuf_tensor(f"ble{i}", [128, ECOLS], f8)) for i in range(2)]
        BLS = ctx.enter_context(nc.sbuf_tensor("bls", [128, SCOLS], bf16))
        BIAS = ctx.enter_context(nc.sbuf_tensor("bias", [128, 128], f32))
        JUNK = ctx.enter_context(nc.sbuf_tensor("junk", [128, 544], f8))
        H1E = [ctx.enter_context(nc.sbuf_tensor(f"h1e{i}", [128, KH * CAP_E], f8)) for i in range(2)]
        H1S = ctx.enter_context(nc.sbuf_tensor("h1s", [128, KHS * CAP_S], bf16))
        OTE = [ctx.enter_context(nc.sbuf_tensor(f"ote{i}", [128, KC * CAP_E], f8)) for i in range(2)]
        OTS = ctx.enter_context(nc.sbuf_tensor("ots", [128, KC * CAP_S], bf16))
        PS = [ctx.enter_context(nc.psum_tensor(f"ps{i}", [128, 512], f32)) for i in range(8)]
        inA = [ctx.enter_context(nc.semaphore(f"inA{g}")) for g in range(G)]
        inB = [ctx.enter_context(nc.semaphore(f"inB{g}")) for g in range(G)]
        inC = [ctx.enter_context(nc.semaphore(f"inC_{g}")) for g in range(G)]
        biasS = ctx.enter_context(nc.semaphore("biasS"))
        junkS = ctx.enter_context(nc.semaphore("junkS"))
        outS = ctx.enter_context(nc.semaphore("outS"))
        pe1 = ctx.enter_context(nc.semaphore("pe1"))
        pe2 = ctx.enter_context(nc.semaphore("pe2"))
        actE = ctx.enter_context(nc.semaphore("actE"))
        actO = ctx.enter_context(nc.semaphore("actO"))
        dve1 = ctx.enter_context(nc.semaphore("dve1"))
        dveS = ctx.enter_context(nc.semaphore("dveS"))
        block = ctx.enter_context(nc.Block(no_gpsimd_drain=True))

        def seg(g):
            if g != 1:
                i = 0 if g == 0 else 1
                return (BLE[i][:], H1E[i][:], OTE[i][:], CAP_E, EW1, EW2, True,
                        eout[i], KH)
            return (BLS[:], H1S[:], OTS[:], CAP_S, SW1, SW2, False, sout, KHS)

        if vsplit:
            SCm = [[0, 2, 4, 6], [0, 1], [0, 2, 4, 6]]
            VEm = [[1, 3, 5, 7], [2, 3], [1, 3, 5, 7]]
        else:
            SCm = [list(range(KH)), list(range(KHS)), list(range(KH))]
            VEm = [[], [], []]
        eoff = [sum(len(SCm[x]) for x in range(g)) for g in range(G)]
        ooff = [sum(len(VEm[x]) for x in range(g)) for g in range(G)]
        p1off = [sum(seg(x)[8] for x in range(g)) for g in range(G)]

        def act_wait(stream, g, m):
            if m in SCm[g]:
                stream.wait_ge(actE, eoff[g] + SCm[g].index(m) + 1)
            if m in VEm[g]:
                stream.wait_ge(actO, ooff[g] + VEm[g].index(m) + 1)

        def act_wait_upto(stream, g, mmax):
            se = [m for m in SCm[g] if m <= mmax]
            so = [m for m in VEm[g] if m <= mmax]
            if se:
                stream.wait_ge(actE, eoff[g] + SCm[g].index(se[-1]) + 1)
            if so:
                stream.wait_ge(actO, ooff[g] + VEm[g].index(so[-1]) + 1)

        @block.sync
        def _(sync):
            for g in range(G):
                bl, _h1, _ot, cap, w1o, w2o, _f, _od, nh1 = seg(g)
                src = sblob if g == 1 else eblob[0 if g == 0 else 1]
                w1cols = w2o - w1o
                w2cols = nh1 * C if g != 1 else KHS * C
                a_end = w1o + w1cols // 2
                c_beg = w2o + w2cols // 2
                cols = w2o + w2cols
                sync.dma_start(out=bl[:, 0:a_end], in_=src[:, 0:a_end]).then_inc(inA[g], 16)
                sync.dma_start(out=bl[:, a_end:c_beg], in_=src[:, a_end:c_beg]).then_inc(inB[g], 16)
                sync.dma_start(out=bl[:, c_beg:cols], in_=src[:, c_beg:cols]).then_inc(inC[g], 16)
            for g in range(G):
                _bl, _h1, ot, cap, _w1o, _w2o, _f, od, _n = seg(g)
                sync.wait_ge(dve1, 2 * g + 2)
                sync.dma_start(
                    out=od[:, 0: 2 * cap], in_=ot[:, 0: 2 * cap]
                ).then_inc(outS, 16)
                sync.wait_ge(dveS, 2 * g + 2)
                sync.dma_start(
                    out=od[:, 2 * cap: 4 * cap], in_=ot[:, 2 * cap: 4 * cap]
                ).then_inc(outS, 16)
            sync.wait_ge(outS, 16 * 2 * G)

        @block.gpsimd
        def _(gpsimd):
            nc.gpsimd.memset(JUNK[:], 0).then_inc(junkS, 1)
            gpsimd.dma_start(out=BIAS[:], in_=biasb[:, :]).then_inc(biasS, 16)

        @block.tensor
        def _(tensor):
            tensor.wait_ge(junkS, 1)
            for _ in range(30):
                nc.tensor.matmul(
                    PS[7][:, :128],
                    lhsT=JUNK[:, 0:256].rearrange("p (two f) -> p two f", two=2),
                    rhs=JUNK[:, 256:512].rearrange("p (two f) -> p two f", two=2),
                    start=True,
                    stop=True,
                    perf_mode=DR,
                )
            for g in range(G):
                bl, h1, _ot, cap, w1o, w2o, fp8, _od, nh1 = seg(g)
                tensor.wait_ge(inA[g], 16)
                for m in range(nh1):
                    if m == nh1 // 2:
                        tensor.wait_ge(inB[g], 16)
                    if m >= 4 and m % 2 == 0:
                        act_wait(tensor, g, m - 4 + 1)
                        act_wait(tensor, g, m - 4)
                    if fp8:
                        for j in range(KC // 2):
                            mm = nc.tensor.matmul(
                                PS[m % 4][:, :cap],
                                lhsT=bl[:, w1o + m * 512 + j * 256: w1o + m * 512 + (j + 1) * 256]
                                .rearrange("p (two f) -> p two f", two=2),
                                rhs=bl[:, 2 * j * cap: (2 * j + 2) * cap]
                                .rearrange("p (two f) -> p two f", two=2),
                                start=(j == 0),
                                stop=(j == KC // 2 - 1),
                                perf_mode=DR,
                            )
                    else:
                        for k in range(KC):
                            mm = nc.tensor.matmul(
                                PS[m % 4][:, :cap],
                                lhsT=bl[:, w1o + m * 512 + k * 128: w1o + m * 512 + (k + 1) * 128],
                                rhs=bl[:, k * cap: (k + 1) * cap],
                                start=(k == 0),
                                stop=(k == KC - 1),
                            )
                    mm.then_inc(pe1, 1)
                if fp8:
                    for j in range(KH // 2):
                        if j == 2:
                            tensor.wait_ge(inC[g], 16)
                        if j % 2 == 0:
                            act_wait_upto(tensor, g, 2 * j + 3)
                        for m2 in range(KC):
                            if j == 0 and g >= 1:
                                if m2 < 2:
                                    tensor.wait_ge(dve1, 2 * (g - 1) + m2 + 1)
                                else:
                                    tensor.wait_ge(dveS, 2 * (g - 1) + m2 - 1)
                            mm = nc.tensor.matmul(
                                PS[4 + m2][:, :cap],
                                lhsT=bl[:, w2o + j * 1024 + m2 * 256: w2o + j * 1024 + m2 * 256 + 256]
                                .rearrange("p (two f) -> p two f", two=2),
                                rhs=h1[:, 2 * j * cap: (2 * j + 2) * cap]
                                .rearrange("p (two f) -> p two f", two=2),
                                start=(j == 0),
                                stop=(j == KH // 2 - 1),
                                perf_mode=DR,
                            )
                            if j == KH // 2 - 1:
                                mm.then_inc(pe2, 1)
                else:
                    for k2 in range(KHS):
                        if k2 == KHS // 2:
                            tensor.wait_ge(inC[g], 16)
                        if k2 % 2 == 0:
                            act_wait_upto(tensor, g, k2 + 1)
                        for m2 in range(KC):
                            if k2 == 0 and g >= 1:
                                if m2 < 2:
                                    tensor.wait_ge(dve1, 2 * (g - 1) + m2 + 1)
                                else:
                                    tensor.wait_ge(dveS, 2 * (g - 1) + m2 - 1)
                            mm = nc.tensor.matmul(
                                PS[4 + m2][:, :cap],
                                lhsT=bl[:, w2o + k2 * 512 + m2 * 128: w2o + k2 * 512 + (m2 + 1) * 128],
                                rhs=h1[:, k2 * cap: (k2 + 1) * cap],
                                start=(k2 == 0),
                                stop=(k2 == KHS - 1),
                            )
                            if k2 == KHS - 1:
                                mm.then_inc(pe2, 1)

        @block.scalar
        def _(scalar):
            import concourse.mybir as mybir

            scalar.wait_ge(junkS, 1)
            nc.scalar.activation(
                JUNK[:, 528:544], JUNK[:, 0:16],
                mybir.ActivationFunctionType.Relu,
            )
            nc.scalar.activation(
                JUNK[:, 512:528], JUNK[:, 0:16],
                mybir.ActivationFunctionType.Identity,
            )
            scalar.wait_ge(biasS, 16)

            def s_act(g, m):
                _b, h1, _o, cap, _w, _w2, f8g, _d, _n = seg(g)
                scalar.wait_ge(pe1, p1off[g] + m + 1)
                nc.scalar.activation(
                    h1[:, m * cap: (m + 1) * cap],
                    PS[m % 4][:, :cap],
                    mybir.ActivationFunctionType.Relu,
                    bias=BIAS[:, g * 12 + m: g * 12 + m + 1],
                    scale=(SHS / (SX * S1)) if f8g else 1.0,
                ).then_inc(actE, 1)

            def s_out(g, m2):
                _b, _h, ot, cap, _w, _w2, f8g, _d, _n = seg(g)
                scalar.wait_ge(pe2, 4 * g + m2 + 1)
                nc.scalar.activation(
                    ot[:, m2 * cap: (m2 + 1) * cap],
                    PS[4 + m2][:, :cap],
                    mybir.ActivationFunctionType.Identity,
                    bias=BIAS[:, g * 12 + 8 + m2: g * 12 + 8 + m2 + 1],
                    scale=(SO / (SHS * S2)) if f8g else 1.0,
                ).then_inc(dveS, 1)

            seq = []
            for g in range(G):
                seq += [("a", g, m) for m in SCm[g]]
                seq += [("o", g, 2), ("o", g, 3)]
            if vsplit:
                seq.remove(("a", 2, SCm[2][0]))
                seq.insert(seq.index(("o", 1, 3)), ("a", 2, SCm[2][0]))
            for kind, g, m in seq:
                (s_act if kind == "a" else s_out)(g, m)

        @block.vector
        def _(vector):
            import concourse.mybir as mybir

            vector.wait_ge(biasS, 16)

            def v_act(g, m):
                _b, h1, _o, cap, _w, _w2, f8g, _d, _n = seg(g)
                vector.wait_ge(pe1, p1off[g] + m + 1)
                nc.vector.tensor_scalar(
                    h1[:, m * cap: (m + 1) * cap],
                    PS[m % 4][:, :cap],
                    (SHS / (SX * S1)) if f8g else 1.0,
                    0.0,
                    mybir.AluOpType.mult,
                    mybir.AluOpType.max,
                ).then_inc(actO, 1)

            def v_out(g, m2):
                _b, _h, ot, cap, _w, _w2, f8g, _d, _n = seg(g)
                vector.wait_ge(pe2, 4 * g + m2 + 1)
                nc.vector.tensor_scalar(
                    ot[:, m2 * cap: (m2 + 1) * cap],
                    PS[4 + m2][:, :cap],
                    (SO / (SHS * S2)) if f8g else 1.0,
                    BIAS[:, g * 12 + 8 + m2: g * 12 + 8 + m2 + 1],
                    mybir.AluOpType.mult,
                    mybir.AluOpType.add,
                ).then_inc(dve1, 1)

            seqv = []
            for g in range(G):
                seqv += [("a", g, m) for m in VEm[g]]
                seqv += [("o", g, 0), ("o", g, 1)]
            if vsplit:
                seqv.remove(("a", 2, VEm[2][0]))
                seqv.insert(seqv.index(("o", 1, 1)), ("a", 2, VEm[2][0]))
            for kind, g, m in seqv:
                (v_act if kind == "a" else v_out)(g, m)

    return nc


def _route(x2, bucket, expert_key):
    """Host router in float64. Returns gid (N,2), combine weights (N,2)."""
    hn = x2 / np.maximum(np.linalg.norm(x2, axis=-1, keepdims=True), 1e-12)
    keys = expert_key / np.maximum(
        np.linalg.norm(expert_key, axis=-1, keepdims=True), 1e-12
    )
    kb = keys[bucket]  # (N, EPB, C)
    score = np.einsum("nc,nec->ne", hn, kb) / max(TAU, 1e-6)
    score -= score.max(axis=-1, keepdims=True)
    p = np.exp(score)
    p /= p.sum(axis=-1, keepdims=True)
    local = np.argsort(-p, axis=-1, kind="stable")[:, :TOPK]  # (N, 2)
    topv = np.take_along_axis(p, local, axis=-1)
    w = topv / (topv.sum(axis=-1, keepdims=True) + 1e-9)
    gid = bucket[:, None] * EPB + local
    return gid, w


def _wcols(w_, kin, scale, qdt):
    """(kin*128, kout*128) weight -> [128, kout*kin*128] m-major blob cols."""
    kout = w_.shape[1] // 128
    wq = (np.asarray(w_, np.float32) * scale).astype(qdt)
    return wq.reshape(kin, 128, kout, 128).transpose(1, 2, 0, 3).reshape(128, kout * kin * 128)


def _w2cols_e(w_, scale):
    """Expert W2 (H, C) -> fp8 [128, 4096], col j*1024 + m2*256 + i*128 + c."""
    wq = (np.asarray(w_, np.float32) * scale).astype(F8)
    return wq.reshape(KH // 2, 2, 128, KC, 128).transpose(2, 0, 3, 1, 4).reshape(128, KH * C)


def _w2cols_s(w_):
    """Shared W2 half (512, C) -> bf16 [128, 2048], col k2*512 + m2*128 + c."""
    wq = np.asarray(w_, np.float32).astype(BF16)
    return wq.reshape(KHS, 128, KC, 128).transpose(1, 0, 2, 3).reshape(128, KHS * C)


def kernel(**inputs):
    from concourse.bass_utils import run_bass_kernel_spmd

    x = np.asarray(inputs["x"], dtype=np.float32)
    op_id = np.asarray(inputs["op_id"]).astype(np.int64)
    expert_key = np.asarray(inputs["expert_key"], dtype=np.float64)
    sW1 = np.asarray(inputs["sW1"], dtype=np.float32)
    sb1 = np.asarray(inputs["sb1"], dtype=np.float32)
    sW2 = np.asarray(inputs["sW2"], dtype=np.float32)
    sb2 = np.asarray(inputs["sb2"], dtype=np.float32)
    eW1 = np.asarray(inputs["eW1"], dtype=np.float32)
    eb1 = np.asarray(inputs["eb1"], dtype=np.float32)
    eW2 = np.asarray(inputs["eW2"], dtype=np.float32)
    eb2 = np.asarray(inputs["eb2"], dtype=np.float32)
    gate_logit = float(np.asarray(inputs["gate_logit"]))

    B, T, Cc = x.shape
    assert Cc == C
    N = B * T
    assert N == CAP_S * (N_CORES // 2)
    x2 = x.reshape(N, C)
    bucket = np.clip(op_id.reshape(-1), 0, N_BUCKET - 1)

    gid, w = _route(x2.astype(np.float64), bucket, expert_key)
    gate = 1.0 / (1.0 + np.exp(-gate_logit))

    flat_gid = gid.reshape(-1)  # (N*2,) ; slot i -> token i//2
    sorted_slots = np.argsort(flat_gid, kind="stable")
    counts = np.bincount(flat_gid, minlength=E)

    CAP_E = max(64, -(-int(counts.max()) // 4) * 4)
    assert CAP_E <= 512, CAP_E
    EXT, EW1c, EW2c, ECOLS = _elayout(CAP_E)

    fast = bool(
        np.all(eb1 == 0) and np.all(sb1 == 0)
        and np.all(eb2 == 0) and np.all(sb2 == 0)
    )
    vsplit = bool(np.all(eb1 == 0) and np.all(sb1 == 0))

    eblob = np.zeros((N_CORES, 2, 128, ECOLS), F8)
    sblob = np.zeros((N_CORES, 128, SCOLS), BF16)
    biasb = np.zeros((N_CORES, 128, 128), np.float32)
    slot_flat = np.zeros((2, N), np.int64)

    x2T = np.ascontiguousarray(x2.T)  # (C, N)
    xq8 = (x2T * SX).astype(F8)
    xb16 = x2T.astype(BF16)

    pos = 0
    for e in range(E):
        cnt = int(counts[e])
        chunk = sorted_slots[pos: pos + cnt]
        pos += cnt
        c, i = e // 2, e % 2
        w1c = _wcols(eW1[e], KC, S1, F8)
        if i == 1 and fast:
            # E1's mm1 issues m in order [4..7, 0..3]; pack W1 to match
            w1c = np.ascontiguousarray(
                w1c.reshape(128, KH, KC * 128)[:, [4, 5, 6, 7, 0, 1, 2, 3], :]
            ).reshape(128, KH * KC * 128)
        eblob[c, i, :, EW1c:EW2c] = w1c
        eblob[c, i, :, EW2c:ECOLS] = _w2cols_e(eW2[e], S2)
        toks = chunk // TOPK
        eblob[c, i, :, EXT:EW1c].reshape(128, KC, CAP_E)[:, :, :cnt] = (
            xq8[:, toks].reshape(KC, 128, cnt).transpose(1, 0, 2)
        )
        if not fast:
            gb = 0 if i == 0 else 24  # device segment order: E0, shared, E1
            biasb[c, :, gb: gb + KH] = SHS * eb1[e].reshape(KH, 128).T
            biasb[c, :, gb + 8: gb + 8 + KC] = SO * eb2[e].reshape(KC, 128).T
        slot_flat[chunk % TOPK, toks] = (2 * c + i) * CAP_E + np.arange(cnt)

    for c in range(N_CORES):
        p, half = c // 2, c % 2
        lo = p * CAP_S
        hcols = slice(half * (H // 2), (half + 1) * (H // 2))
        sblob[c, :, SW1:SW2] = _wcols(sW1[:, hcols], KC, 1.0, BF16)
        sblob[c, :, SW2:SCOLS] = _w2cols_s(sW2[hcols, :])
        sblob[c, :, :SW1] = (
            xb16[:, lo: lo + CAP_S].reshape(KC, 128, CAP_S).transpose(1, 0, 2)
            .reshape(128, KC * CAP_S)
        )
        if not fast:
            biasb[c, :, 12: 12 + KHS] = sb1[hcols].reshape(KHS, 128).T
            if half == 0:  # b2 added by one core of the pair only
                biasb[c, :, 20: 20 + KC] = sb2.reshape(KC, 128).T

    key = (CAP_E, "fast") if fast else (CAP_E, vsplit)
    if key not in _BUILD_CACHE:
        _BUILD_CACHE[key] = (
            _build_program_fast(CAP_E) if fast
            else _build_program_biased(CAP_E, vsplit)
        )
    nc = _BUILD_CACHE[key]

    if fast:
        in_maps = [
            {"eblob": eblob[c], "sblob": sblob[c]} for c in range(N_CORES)
        ]
    else:
        in_maps = [
            {"eblob": eblob[c], "sblob": sblob[c], "biasb": biasb[c]}
            for c in range(N_CORES)
        ]

    import os

    trace = bool(os.environ.get("BASS_TRACE"))
    res = run_bass_kernel_spmd(
        nc,
        in_maps,
        core_ids=list(range(N_CORES)),
        trace=trace,
        trace_cores=list(range(N_CORES)) if trace else None,
    )
    global LAST_EXEC_NS, LAST_RESULTS
    LAST_EXEC_NS = res.exec_time_ns
    LAST_RESULTS = res

    # un-shard: col m2*cap+t, C index = m2*128+p -> token-major rows
    eallout = np.empty((N_CORES * 2 * CAP_E, C), np.float32)
    sallout = np.zeros((N, C), np.float32)
    for c in range(N_CORES):
        eo = np.asarray(res.results[c]["eout"]).astype(np.float32) / SO
        eallout[c * 2 * CAP_E: (c + 1) * 2 * CAP_E] = (
            eo.reshape(2, 128, KC, CAP_E).transpose(0, 3, 2, 1).reshape(2 * CAP_E, C)
        )
        so = np.asarray(res.results[c]["sout"]).astype(np.float32)
        lo = (c // 2) * CAP_S
        sallout[lo: lo + CAP_S] += (
            so.reshape(128, KC, CAP_S).transpose(2, 1, 0).reshape(CAP_S, C)
        )

    wf = (gate * w).astype(np.float32)  # (N, 2) combine weights
    y = (
        eallout[slot_flat[0]] * wf[:, 0:1]
        + eallout[slot_flat[1]] * wf[:, 1:2]
        + sallout
    )
    return y.reshape(B, T, C)


LAST_EXEC_NS = None
LAST_RESULTS = None
